# revision 1
# baseline (speedup 1.0000x reference)
"""Trainium2 Bass kernel for the inverse deep-hough-transform gather-reduce.

out[n, c, y, x] = sum_k acc[n, c, k, rho_idx[k, y, x]]  (masked by validity)

Design (v2)
-----------
- The rho index table is a pure function of static shapes; precomputed on the
  host.  Validity is folded in by pointing invalid entries at a zeroed spare
  slot (block R) of the data table.
- Gather primitive: GPSIMD IndirectCopy.  Measured cost is per 3-index
  read-request (~102-cycle serial RD_CMD), so each index fetches a contiguous
  block of F=32 nc-values -> 3.6us per 32-index IC (1024 elems/partition, the
  ISA cap per IC).
- Sharding: every core holds ALL 512 (n,c) rows; angles are sharded as
  k === core (mod 8).  Host sums the 8 per-core partials (the unshard step).
- Layout: 128 partitions = 8 groups x 16.  At step s, group g processes angle
  slot s*8+g (3 steps x 8 groups = 24 slots >= 23 angles/core; extra slots
  gather zeros).  Partition p = g*16+u plus block lane f in [0,32) covers
  nc = f*16+u.
- Per yx-chunk (64 positions): GPSIMD gathers [128, 2048] per step, DVE
  accumulates over steps, PE reduces the 8 groups with an exact 0/1 fp32
  selection matmul into PSUM, ACT copies PSUM->SBUF, sync DMA dumps to HBM.
- Raw Bass blocks with explicit semaphores (this walrus build allows at most
  one attached sync-wait per engine instruction; standalone EVSEM waits are
  used instead).
"""

from contextlib import ExitStack

import numpy as np

import concourse.bass as bass
from concourse import mybir
from concourse.bass_utils import run_bass_kernel_spmd

# Problem constants (hardcoded per the harness contract).
N, C, A, R = 4, 128, 180, 184
H = W = 128
YX = H * W  # 16384
NCORES = 8

GROUPS = 8  # 16-partition groups
U = 16  # partitions per group
F = 32  # nc values packed per rho block (IC inner size)
NCPC = F * U  # 512 nc rows held per core (all of them)
STEPS = 3  # angle slots per core = STEPS*GROUPS = 24 >= ceil(180/8)
ASLOT = STEPS * GROUPS  # 24
RPAD = R + 1  # 185 blocks; block R is all-zeros (invalid sink)
CHUNK = 64  # yx positions per chunk
NCH = YX // CHUNK  # 256 chunks
DW = RPAD * F  # data words per step per partition (5920)
CW = CHUNK // U  # idx columns per (step, chunk) per partition (4)
ICSUB = 1024 // F  # indices per IC (32): ISA caps IC dst at 1024 elems
NSUB = CHUNK // ICSUB  # sub-ICs per (chunk, step) (2)

_cache = {}


def _core_angles(core):
    """Angle slots for this core: slot t (0..23) -> global k or None."""
    ks = [k for k in range(A) if k % NCORES == core]
    return [ks[t] if t < len(ks) else None for t in range(ASLOT)]


def _rho_block_table():
    """[A, YX] int32 block indices into the padded rho axis (R = invalid)."""
    if "blk" in _cache:
        return _cache["blk"]
    k = np.arange(A)
    theta = k * (np.pi / A)
    cos_t = np.cos(theta)
    sin_t = np.sin(theta)
    y, x = np.meshgrid(np.arange(H), np.arange(W), indexing="ij")
    xc = (x - W // 2).astype(np.float64)
    yc = (y - H // 2).astype(np.float64)
    r = np.round(cos_t[:, None, None] * xc[None] + sin_t[:, None, None] * yc[None])
    r = r.astype(np.int64) + R // 2  # [A, H, W]
    valid = (r >= 0) & (r < R)
    blk = np.where(valid, np.clip(r, 0, R - 1), R).astype(np.int32)
    _cache["blk"] = blk.reshape(A, YX)
    return _cache["blk"]


def _idx_table(core):
    """uint16 idx stream for one core, SBUF layout [128, STEPS*NCH*CW].

    IndirectCopy unwraps a 16-partition group's idx tile as
    (col*16 + p_in_group); values are flat element offsets = block*F.
    Column layout: col = s*(NCH*CW) + q*CW + w.
    """
    key = ("idx", core)
    if key in _cache:
        return _cache[key]
    blk = _rho_block_table()
    angles = _core_angles(core)
    out = np.full((128, STEPS * NCH * CW), R * F, np.uint16)
    for s in range(STEPS):
        for g in range(GROUPS):
            k = angles[s * GROUPS + g]
            if k is None:
                continue
            flat = (blk[k] * F).astype(np.uint16)  # [YX]
            v = flat.reshape(NCH, CW, U)  # [q, w, p_in_group]
            v = v.transpose(2, 0, 1).reshape(U, NCH * CW)
            cols = slice(s * NCH * CW, (s + 1) * NCH * CW)
            out[g * U : (g + 1) * U, cols] = v
    _cache[key] = out
    return out


def _data_for_core(acc_flat, core):
    """acc_flat [512, A, R] f32 -> [128, STEPS*DW] f32 padded gather table.

    data[g*16+u, s*DW + rb*F + f] = acc_flat[f*16+u, k(s*8+g), rb]
    """
    angles = _core_angles(core)
    ac = np.zeros((NCPC, ASLOT, RPAD), np.float32)
    for t, k in enumerate(angles):
        if k is not None:
            ac[:, t, :R] = acc_flat[:, k, :]
    t = ac.reshape(F, U, ASLOT, RPAD)  # [f, u, t, r]
    t = t.transpose(2, 1, 3, 0)  # [t, u, r, f]
    t = t.reshape(STEPS, GROUPS, U, RPAD, F)  # [s, g, u, r, f]
    t = t.reshape(STEPS, 128, DW)
    return np.ascontiguousarray(t.transpose(1, 0, 2).reshape(128, STEPS * DW))


def _sel_matrix():
    """[128, 16] f32 selection: S[p, m] = 1 if p % 16 == m (group reduce)."""
    s = np.zeros((128, U), np.float32)
    s[np.arange(128), np.arange(128) % U] = 1.0
    return s


def _build_nc():
    if "nc" in _cache:
        return _cache["nc"]
    nc = bass.Bass("TRN2", debug=False, target_bir_lowering=False, num_devices=NCORES)
    data_d = nc.dram_tensor(
        "data", [128, STEPS * DW], mybir.dt.float32, kind="ExternalInput"
    ).ap()
    idx_d = nc.dram_tensor(
        "idx", [128, STEPS * NCH * CW], mybir.dt.uint16, kind="ExternalInput"
    ).ap()
    sel_d = nc.dram_tensor(
        "sel", [128, U], mybir.dt.float32, kind="ExternalInput"
    ).ap()
    raw_d = nc.dram_tensor(
        "raw", [NCH, U, CHUNK * F], mybir.dt.float32, kind="ExternalOutput"
    ).ap()

    GW = CHUNK * F  # 2048 gather/accum words per partition per (chunk, step)
    NMM = GW // 512  # matmuls per chunk (PSUM bank = 512 fp32)

    ctx = ExitStack()
    _cache["ctx"] = ctx
    data_sb = ctx.enter_context(nc.sbuf_tensor("data_sb", [128, STEPS * DW], mybir.dt.float32))
    idx_sb = ctx.enter_context(
        nc.sbuf_tensor("idx_sb", [128, STEPS * NCH * CW], mybir.dt.uint16)
    )
    sel_sb = ctx.enter_context(nc.sbuf_tensor("sel_sb", [128, U], mybir.dt.float32))
    NBUF = 4
    gbuf = [
        ctx.enter_context(nc.sbuf_tensor(f"gbuf{i}", [128, GW], mybir.dt.float32))
        for i in range(NBUF)
    ]
    abuf = [
        ctx.enter_context(nc.sbuf_tensor(f"abuf{i}", [128, GW], mybir.dt.float32))
        for i in range(4)
    ]
    obuf = [
        ctx.enter_context(nc.sbuf_tensor(f"obuf{i}", [U, GW], mybir.dt.float32))
        for i in range(2)
    ]
    psum = [
        ctx.enter_context(nc.psum_tensor(f"ps{i}", [U, GW], mybir.dt.float32))
        for i in range(2)
    ]
    ld_sem = ctx.enter_context(nc.semaphore("ld_sem"))
    ic_sem = ctx.enter_context(nc.semaphore("ic_sem"))
    add_sem = ctx.enter_context(nc.semaphore("add_sem"))
    mm_sem = ctx.enter_context(nc.semaphore("mm_sem"))
    cp_sem = ctx.enter_context(nc.semaphore("cp_sem"))
    dump_sem = ctx.enter_context(nc.semaphore("dump_sem"))
    block = ctx.enter_context(nc.Block())

    @block.gpsimd
    def _(gpsimd):
        gpsimd.dma_start(data_sb[:], data_d[:]).then_inc(ld_sem, 16)
        gpsimd.dma_start(idx_sb[:], idx_d[:]).then_inc(ld_sem, 16)
        gpsimd.dma_start(sel_sb[:], sel_d[:]).then_inc(ld_sem, 16)
        gpsimd.wait_ge(ld_sem, 48)
        jg = 0  # gather-tile slot counter (one per (chunk, step))
        for q in range(NCH):
            for s in range(STEPS):
                # gbuf slot reuse: PE must have consumed slot jg-NBUF.
                if jg >= NBUF:
                    gpsimd.wait_ge(mm_sem, jg - NBUF + 1)
                dst = gbuf[jg % NBUF]
                jg += 1
                dslice = data_sb[:, s * DW : (s + 1) * DW]
                ibase = s * NCH * CW + q * CW
                for sub in range(NSUB):
                    cw2 = CW // NSUB  # idx cols per sub-IC
                    isl = idx_sb[
                        :, ibase + sub * cw2 : ibase + (sub + 1) * cw2
                    ]
                    gpsimd.indirect_copy(
                        out=dst[
                            :, sub * (GW // NSUB) : (sub + 1) * (GW // NSUB)
                        ].rearrange("p (i f) -> p i f", f=F),
                        data=dslice.rearrange("p (r f) -> p r f", f=F),
                        idxs=isl,
                        i_know_ap_gather_is_preferred=True,
                    ).then_inc(ic_sem, 1)

    @block.tensor
    def _(tensor):
        # PE does the cross-step accumulation in PSUM (its own SBUF ports,
        # so the gather stream sees zero Pool-port contention from it).
        jg = 0
        for q in range(NCH):
            if q >= 2:
                tensor.wait_ge(cp_sem, q - 1)  # psum slot reused
            for s in range(STEPS):
                tensor.wait_ge(ic_sem, (q * STEPS + s + 1) * NSUB)
                for m in range(NMM):
                    mm = tensor.matmul(
                        out=psum[q % 2][:, m * 512 : (m + 1) * 512],
                        lhsT=sel_sb[:],
                        rhs=gbuf[jg % NBUF][:, m * 512 : (m + 1) * 512],
                        start=(s == 0),
                        stop=(s == STEPS - 1),
                    )
                    if m == NMM - 1:
                        mm.then_inc(mm_sem, 1)  # counts (q, s) groups
                jg += 1

    @block.scalar
    def _(scalar):
        for q in range(NCH):
            scalar.wait_ge(mm_sem, (q + 1) * STEPS)
            if q >= 2:
                scalar.wait_ge(dump_sem, (q - 1) * 16)  # obuf slot reused
            scalar.copy(obuf[q % 2][:], psum[q % 2][:]).then_inc(cp_sem, 1)

    @block.sync
    def _(sync):
        for q in range(NCH):
            sync.wait_ge(cp_sem, q + 1)
            sync.dma_start(raw_d[q], obuf[q % 2][:]).then_inc(dump_sem, 16)

    _cache["nc"] = nc
    return nc


def _install_ntff_hook():
    """Provide the antenv.axon_hooks shim the image lacks, wiring the
    ctypes NTFF profiler from trn_agent_boot."""
    import sys
    import types

    if "antenv.axon_hooks" in sys.modules:
        return
    import antenv
    from trn_agent_boot.trn_boot import _ntff_profile_via_ctypes

    mod = types.ModuleType("antenv.axon_hooks")
    hook = _ntff_profile_via_ctypes("/opt/axon/libaxon_pjrt.so")
    mod.get_axon_ntff_profile_hook = lambda: hook
    mod.set_axon_ntff_profile_hook = lambda h: None
    sys.modules["antenv.axon_hooks"] = mod
    antenv.axon_hooks = mod


def hw_exec_time_ns(trace_cores=None):
    """Re-run the last kernel() invocation with tracing; return max core ns."""
    _install_ntff_hook()
    nc = _cache["nc"]
    res = run_bass_kernel_spmd(
        nc,
        _cache["in_maps"],
        core_ids=list(range(NCORES)),
        trace=True,
        trace_cores=trace_cores,
    )
    _cache["trace"] = res
    return res.exec_time_ns


def kernel(accumulator, out_H=128, out_W=128, numangle=180, numrho=184):
    accumulator = np.asarray(accumulator, np.float32)
    assert accumulator.shape == (N, C, A, R), accumulator.shape
    assert int(out_H) == H and int(out_W) == W
    assert int(numangle) == A and int(numrho) == R

    nc = _build_nc()
    acc_flat = np.ascontiguousarray(accumulator.reshape(N * C, A, R))
    sel = _sel_matrix()
    in_maps = [
        {
            "data": _data_for_core(acc_flat, core),
            "idx": _idx_table(core),
            "sel": sel,
        }
        for core in range(NCORES)
    ]
    _cache["in_maps"] = in_maps
    res = run_bass_kernel_spmd(nc, in_maps, core_ids=list(range(NCORES)))

    # Unshard: sum the 8 per-core partials.
    # raw[q, u, i*F + f] = partial for nc = f*16+u, yx = q*CHUNK+i
    total = np.zeros((NCPC, YX), np.float64)
    for core in range(NCORES):
        raw = res.results[core]["raw"]  # [NCH, U, CHUNK*F]
        oc = raw.reshape(NCH, U, CHUNK, F).transpose(3, 1, 0, 2).reshape(NCPC, YX)
        total += oc
    return total.astype(np.float32).reshape(N, C, H, W)



# revision 3
# speedup vs baseline: 8.7098x; 8.7098x over previous
"""Trainium2 Bass kernel for the inverse deep-hough-transform gather-reduce.

out[n, c, y, x] = sum_k acc[n, c, k, rho_idx[k, y, x]]

Design (v3): one-hot selection matmuls on the PE
------------------------------------------------
For a 32x32 output tile and angle k, rho_idx spans a band of at most
31*(|sin|+|cos|)+2 <= 46 consecutive rho values.  The per-angle
gather-reduce over a tile is therefore a small-contraction matmul with a
0/1 one-hot selection matrix:

    psum[c, col] += sum_p acc[n, c, k, base_kt + p] * Sel_kt[p, col]

where Sel_kt[p, col] = 1[rho_idx(k, y(col), x(col)) - base_kt == p].
Two angles are stacked per matmul (K = 2*48 = 96 <= 128) and all 180
angles accumulate in PSUM (fp32).  The acc bands (lhsT) and the one-hot
tables (rhs) are layout-prepped host-side (pure static re-indexing of
the input + 0/1 tables) and streamed from HBM as one combined tensor.

Sharding: core = 2*n + yhalf (N=4 samples x 2 y-halves).  Each core
computes out[n, :, yh*64:(yh+1)*64, :] -- full inputs, disjoint outputs,
no cross-core reduction.  Per core: 8 tiles x 90 superangles x 2 halves
= 1440 matmuls of [K=96] x [128 x 512].

Sync note: a dma_start's completion semaphore gets +16 spread across the
DMA engines as sub-streams finish, so increments of back-to-back DMAs on
one semaphore interleave; waiting for 16*(j+1) on a shared semaphore
does NOT guarantee DMA j finished.  Each ring slot therefore gets its
own semaphore, and slot reuse is gated on the consumer (so increments of
different uses of one slot cannot overlap in time).
"""

from contextlib import ExitStack

import numpy as np
import ml_dtypes

import concourse.bass as bass
from concourse import mybir
from concourse.bass_utils import run_bass_kernel_spmd

# Problem constants (hardcoded per the harness contract).
N, C, A, R = 4, 128, 180, 184
H = W = 128
NCORES = 8

KB = 48  # band rows per angle (>= max band width 46)
STACK = 2  # angles stacked per matmul
K = KB * STACK  # 96 contraction rows
SUPER = A // STACK  # 90 superangles
TS = 32  # tile side
NTY, NTX = 2, 4  # tiles per core: 2 (y within half) x 4 (x)
NT = NTY * NTX  # 8 tiles
COLS = TS * TS  # 1024 columns per tile
MMH = 2  # matmul column halves (512 each, PSUM bank limit)
G = 15  # superangles per DMA batch
NB = SUPER // G  # 6 batches per tile
NJ = NT * NB  # 48 batches per core
RING = 4  # input ring depth
BW = G * (C + COLS)  # combined batch width (lhsT block + rhs block)

BF16 = ml_dtypes.bfloat16

_cache = {}


def _r_table():
    """[A, H, W] int16 rho indices, exactly as the reference computes them."""
    if "rtab" in _cache:
        return _cache["rtab"]
    k = np.arange(A)
    theta = k * (np.pi / A)
    cos_t = np.cos(theta)
    sin_t = np.sin(theta)
    y, x = np.meshgrid(np.arange(H), np.arange(W), indexing="ij")
    xc = (x - W // 2).astype(np.float64)
    yc = (y - H // 2).astype(np.float64)
    r = np.round(cos_t[:, None, None] * xc[None] + sin_t[:, None, None] * yc[None])
    r = r.astype(np.int64) + R // 2
    assert (r >= 0).all() and (r < R).all()  # always valid for these shapes
    _cache["rtab"] = r.astype(np.int16)
    return _cache["rtab"]


def _bases():
    """[A, 4, 4] int16 band base per (angle, global ty, tx); width <= KB."""
    if "bases" in _cache:
        return _cache["bases"]
    r = _r_table().reshape(A, 4, TS, 4, TS)
    rmin = r.min(axis=(2, 4))  # [A, 4ty, 4tx]
    rmax = r.max(axis=(2, 4))
    assert int((rmax - rmin).max()) < KB, (rmax - rmin).max()
    base = np.minimum(rmin, R - KB).astype(np.int16)
    assert int((rmax - base).max()) < KB
    _cache["bases"] = base
    return _cache["bases"]


def _stab(yh):
    """One-hot rhs tables for y-half yh: [NT, K, SUPER, COLS] bf16.

    stab[t, j*KB+p, sk, col] = 1[ r(2sk+j, y, x) - base == p ]
    with t = ty*4+tx, y = yh*64+ty*32+(col//32), x = tx*32+(col%32).
    """
    key = ("stab", yh)
    if key in _cache:
        return _cache[key]
    r = _r_table()[:, yh * 64 : (yh + 1) * 64, :]  # [A, 64, 128]
    r = r.reshape(A, NTY, TS, NTX, TS).transpose(0, 1, 3, 2, 4).reshape(A, NT, COLS)
    base = _bases()[:, yh * 2 : yh * 2 + 2, :].reshape(A, NT)  # [A, NT]
    rel = (r - base[:, :, None]).astype(np.int16)  # [A, NT, COLS] in [0, KB)
    onehot = rel[:, :, None, :] == np.arange(KB, dtype=np.int16)[None, None, :, None]
    # [A, NT, KB, COLS] -> [SUPER, STACK, NT, KB, COLS] -> [NT, STACK*KB, SUPER, COLS]
    onehot = onehot.reshape(SUPER, STACK, NT, KB, COLS).transpose(2, 1, 3, 0, 4)
    st = np.ascontiguousarray(onehot.reshape(NT, K, SUPER, COLS).astype(BF16))
    _cache[key] = st
    return st


def _ltab(acc, n, yh):
    """Band lhsT tables: [NT, K, SUPER, C] bf16.

    ltab[t, j*KB+p, sk, c] = acc[n, c, 2sk+j, base_kt + p]
    """
    base = _bases()[:, yh * 2 : yh * 2 + 2, :].reshape(A, NT)  # [A, NT]
    idx = base[:, :, None] + np.arange(KB, dtype=np.int16)[None, None]  # [A, NT, KB]
    acc_krc = np.ascontiguousarray(acc[n].transpose(1, 2, 0))  # [A, R, C]
    lt = acc_krc[np.arange(A)[:, None, None], idx]  # [A, NT, KB, C]
    lt = lt.reshape(SUPER, STACK, NT, KB, C).transpose(2, 1, 3, 0, 4)
    return np.ascontiguousarray(lt.reshape(NT, K, SUPER, C).astype(BF16))


def _ctab(acc, n, yh):
    """Combined per-batch stream: [NT, NB, K, BW] bf16.

    Per (tile, batch): cols [0, G*C) hold G lhsT blocks, cols [G*C, BW)
    hold G one-hot rhs blocks.
    """
    lt = _ltab(acc, n, yh).reshape(NT, K, NB, G * C).transpose(0, 2, 1, 3)
    st = _stab(yh).reshape(NT, K, NB, G * COLS).transpose(0, 2, 1, 3)
    return np.ascontiguousarray(np.concatenate([lt, st], axis=3))


def _build_nc():
    if "nc" in _cache:
        return _cache["nc"]
    nc = bass.Bass("TRN2", debug=False, target_bir_lowering=False, num_devices=NCORES)
    ctab_d = nc.dram_tensor(
        "ctab", [NT, NB, K, BW], mybir.dt.bfloat16, kind="ExternalInput"
    ).ap()
    raw_d = nc.dram_tensor(
        "raw", [NT, C, COLS], mybir.dt.float32, kind="ExternalOutput"
    ).ap()

    ctx = ExitStack()
    _cache["ctx"] = ctx
    in_sb = [
        ctx.enter_context(nc.sbuf_tensor(f"in{i}", [K, BW], mybir.dt.bfloat16))
        for i in range(RING)
    ]
    obuf = [
        ctx.enter_context(nc.sbuf_tensor(f"ob{i}", [C, COLS], mybir.dt.float32))
        for i in range(2)
    ]
    psum = [
        ctx.enter_context(nc.psum_tensor(f"ps{i}", [C, COLS], mybir.dt.float32))
        for i in range(4)
    ]
    in_sem = [ctx.enter_context(nc.semaphore(f"in_sem{i}")) for i in range(RING)]
    o_sem = [ctx.enter_context(nc.semaphore(f"o_sem{i}")) for i in range(2)]
    pe_sem = ctx.enter_context(nc.semaphore("pe_sem"))
    cp_sem = ctx.enter_context(nc.semaphore("cp_sem"))
    block = ctx.enter_context(nc.Block())

    @block.gpsimd
    def _(gpsimd):
        for j in range(NJ):
            t, b = divmod(j, NB)
            if j >= RING:
                # slot j%RING free once PE finished batch j-RING
                gpsimd.wait_ge(pe_sem, j - RING + 1)
            gpsimd.dma_start(in_sb[j % RING][:], ctab_d[t, b]).then_inc(
                in_sem[j % RING], 16
            )

    @block.tensor
    def _(tensor):
        for j in range(NJ):
            t, b = divmod(j, NB)
            tensor.wait_ge(in_sem[j % RING], 16 * (j // RING + 1))
            if b == 0 and t >= 4:
                tensor.wait_ge(cp_sem, t - 3)  # psum slot drained
            mm = None
            for kk in range(G):
                sk = b * G + kk
                for h in range(MMH):
                    mm = tensor.matmul(
                        out=psum[t % 4][:, h * 512 : (h + 1) * 512],
                        lhsT=in_sb[j % RING][:, kk * C : (kk + 1) * C],
                        rhs=in_sb[j % RING][
                            :,
                            G * C + kk * COLS + h * 512 : G * C + kk * COLS + (h + 1) * 512,
                        ],
                        start=(sk == 0),
                        stop=(sk == SUPER - 1),
                    )
            mm.then_inc(pe_sem, 1)

    @block.scalar
    def _(scalar):
        for t in range(NT):
            scalar.wait_ge(pe_sem, NB * (t + 1))
            if t >= 2:
                scalar.wait_ge(o_sem[t % 2], 16 * (t // 2))  # obuf slot free
            scalar.copy(obuf[t % 2][:], psum[t % 4][:]).then_inc(cp_sem, 1)

    @block.sync
    def _(sync):
        for t in range(NT):
            sync.wait_ge(cp_sem, t + 1)
            sync.dma_start(raw_d[t], obuf[t % 2][:]).then_inc(o_sem[t % 2], 16)

    _cache["nc"] = nc
    return nc


def _install_ntff_hook():
    """Provide the antenv.axon_hooks shim the image lacks, wiring the
    ctypes NTFF profiler from trn_agent_boot."""
    import sys
    import types

    if "antenv.axon_hooks" in sys.modules:
        return
    import antenv
    from trn_agent_boot.trn_boot import _ntff_profile_via_ctypes

    mod = types.ModuleType("antenv.axon_hooks")
    hook = _ntff_profile_via_ctypes("/opt/axon/libaxon_pjrt.so")
    mod.get_axon_ntff_profile_hook = lambda: hook
    mod.set_axon_ntff_profile_hook = lambda h: None
    sys.modules["antenv.axon_hooks"] = mod
    antenv.axon_hooks = mod


def hw_exec_time_ns(trace_cores=None):
    """Re-run the last kernel() invocation with tracing; return max core ns."""
    _install_ntff_hook()
    nc = _cache["nc"]
    res = run_bass_kernel_spmd(
        nc,
        _cache["in_maps"],
        core_ids=list(range(NCORES)),
        trace=True,
        trace_cores=trace_cores,
    )
    _cache["trace"] = res
    return res.exec_time_ns


def kernel(accumulator, out_H=128, out_W=128, numangle=180, numrho=184):
    accumulator = np.asarray(accumulator, np.float32)
    assert accumulator.shape == (N, C, A, R), accumulator.shape
    assert int(out_H) == H and int(out_W) == W
    assert int(numangle) == A and int(numrho) == R

    nc = _build_nc()
    in_maps = []
    for core in range(NCORES):
        n, yh = divmod(core, 2)
        in_maps.append({"ctab": _ctab(accumulator, n, yh)})
    _cache["in_maps"] = in_maps
    res = run_bass_kernel_spmd(nc, in_maps, core_ids=list(range(NCORES)))

    # Unshard: cores hold disjoint output slabs.
    out = np.empty((N, C, H, W), np.float32)
    for core in range(NCORES):
        n, yh = divmod(core, 2)
        raw = res.results[core]["raw"]  # [NT, C, COLS]
        slab = (
            raw.reshape(NTY, NTX, C, TS, TS)
            .transpose(2, 0, 3, 1, 4)
            .reshape(C, 64, W)
        )
        out[n, :, yh * 64 : (yh + 1) * 64, :] = slab
    return out


# revision 8
# speedup vs baseline: 14.5849x; 1.6745x over previous
"""Trainium2 Bass kernel for the inverse deep-hough-transform gather-reduce.

out[n, c, y, x] = sum_k acc[n, c, k, rho_idx[k, y, x]]

Design (v3): one-hot selection matmuls on the PE
------------------------------------------------
For a 32x32 output tile and angle k, rho_idx spans a band of at most
31*(|sin|+|cos|)+2 <= 46 consecutive rho values.  The per-angle
gather-reduce over a tile is therefore a small-contraction matmul with a
0/1 one-hot selection matrix:

    psum[c, col] += sum_p acc[n, c, k, base_kt + p] * Sel_kt[p, col]

where Sel_kt[p, col] = 1[rho_idx(k, y(col), x(col)) - base_kt == p].
Two angles are stacked per matmul (K = 2*48 = 96 <= 128) and all 180
angles accumulate in PSUM (fp32).  The acc bands (lhsT) and the one-hot
tables (rhs) are layout-prepped host-side (pure static re-indexing of
the input + 0/1 tables) and streamed from HBM as one combined tensor.

fp8 DoubleRow: everything streams as fp8e4 (half the DMA bytes of bf16,
the dominant cost) and the PE runs DoubleRow (0.5 cycles/col).  The
DoubleRow pair dim carries hi = fp8(acc) and res = fp8(acc - hi) against
the same one-hot rhs (stride-0 pair dim), so the matmul computes
(hi + res).T @ Sel -- quantization error ~1e-3, well under bf16 cost.

Sharding: core = 2*n + yhalf (N=4 samples x 2 y-halves).  Each core
computes out[n, :, yh*64:(yh+1)*64, :] -- full inputs, disjoint outputs,
no cross-core reduction.  Per core: 8 tiles x 90 superangles x 2 halves
= 1440 matmuls of [K=96] x [128 x 512].

Sync note: a dma_start's completion semaphore gets +16 spread across the
DMA engines as sub-streams finish, so increments of back-to-back DMAs on
one semaphore interleave; waiting for 16*(j+1) on a shared semaphore
does NOT guarantee DMA j finished.  Each ring slot therefore gets its
own semaphore, and slot reuse is gated on the consumer (so increments of
different uses of one slot cannot overlap in time).
"""

from contextlib import ExitStack

import numpy as np
import ml_dtypes

import concourse.bass as bass
from concourse import mybir
from concourse.bass_utils import run_bass_kernel_spmd

# Problem constants (hardcoded per the harness contract).
N, C, A, R = 4, 128, 180, 184
H = W = 128
NCORES = 8

KB = 48  # band rows per angle (>= max band width 46)
STACK = 2  # angles stacked per matmul
K = KB * STACK  # 96 contraction rows
SUPER = A // STACK  # 90 superangles
TS = 32  # tile side
NTY, NTX = 2, 4  # tiles per core: 2 (y within half) x 4 (x)
NT = NTY * NTX  # 8 tiles
COLS = TS * TS  # 1024 columns per tile
MMH = 2  # matmul column halves (512 each, PSUM bank limit)
LW = 2 * C  # lhsT width per super: fp8 [hi(128) | res(128)]
G = 45  # superangles per DMA batch (57.6KB descriptors, < 64KB cap)
NB = SUPER // G  # 2 batches per tile
NJ = NT * NB  # 16 batches per core
RING = 2  # input ring depth
BW = G * (LW + COLS)  # combined batch width (lhsT blocks + rhs blocks)

BF16 = ml_dtypes.bfloat16
FP8 = ml_dtypes.float8_e4m3

_cache = {}


def _r_table():
    """[A, H, W] int16 rho indices, exactly as the reference computes them."""
    if "rtab" in _cache:
        return _cache["rtab"]
    k = np.arange(A)
    theta = k * (np.pi / A)
    cos_t = np.cos(theta)
    sin_t = np.sin(theta)
    y, x = np.meshgrid(np.arange(H), np.arange(W), indexing="ij")
    xc = (x - W // 2).astype(np.float64)
    yc = (y - H // 2).astype(np.float64)
    r = np.round(cos_t[:, None, None] * xc[None] + sin_t[:, None, None] * yc[None])
    r = r.astype(np.int64) + R // 2
    assert (r >= 0).all() and (r < R).all()  # always valid for these shapes
    _cache["rtab"] = r.astype(np.int16)
    return _cache["rtab"]


def _bases():
    """[A, 4, 4] int16 band base per (angle, global ty, tx); width <= KB."""
    if "bases" in _cache:
        return _cache["bases"]
    r = _r_table().reshape(A, 4, TS, 4, TS)
    rmin = r.min(axis=(2, 4))  # [A, 4ty, 4tx]
    rmax = r.max(axis=(2, 4))
    assert int((rmax - rmin).max()) < KB, (rmax - rmin).max()
    base = np.minimum(rmin, R - KB).astype(np.int16)
    assert int((rmax - base).max()) < KB
    _cache["bases"] = base
    return _cache["bases"]


def _stab(yh):
    """One-hot rhs tables for y-half yh: [NT, K, SUPER, COLS] bf16.

    stab[t, j*KB+p, sk, col] = 1[ r(2sk+j, y, x) - base == p ]
    with t = ty*4+tx, y = yh*64+ty*32+(col//32), x = tx*32+(col%32).
    """
    key = ("stab", yh)
    if key in _cache:
        return _cache[key]
    r = _r_table()[:, yh * 64 : (yh + 1) * 64, :]  # [A, 64, 128]
    r = r.reshape(A, NTY, TS, NTX, TS).transpose(0, 1, 3, 2, 4).reshape(A, NT, COLS)
    base = _bases()[:, yh * 2 : yh * 2 + 2, :].reshape(A, NT)  # [A, NT]
    rel = (r - base[:, :, None]).astype(np.int16)  # [A, NT, COLS] in [0, KB)
    onehot = rel[:, :, None, :] == np.arange(KB, dtype=np.int16)[None, None, :, None]
    # [A, NT, KB, COLS] -> [SUPER, STACK, NT, KB, COLS] -> [NT, STACK*KB, SUPER, COLS]
    onehot = onehot.reshape(SUPER, STACK, NT, KB, COLS).transpose(2, 1, 3, 0, 4)
    st = np.ascontiguousarray(onehot.reshape(NT, K, SUPER, COLS).astype(FP8))
    _cache[key] = st
    return st


def _ltab(acc, n, yh):
    """Band lhsT tables: [NT, K, SUPER, LW] fp8, DoubleRow pair layout.

    Per super: cols [0, C) = hi band, cols [C, 2C) = residual band, where
    hi = fp8(acc), res = fp8(acc - hi); the DoubleRow matmul computes
    hi.T @ Sel + res.T @ Sel = fp8-pair(acc).T @ Sel (~0.1% quantization).

    ltab[t, j*KB+p, sk, {c, C+c}] = {hi, res}[n, c, 2sk+j, base_kt + p]
    """
    base = _bases()[:, yh * 2 : yh * 2 + 2, :].reshape(A, NT)  # [A, NT]
    idx = base[:, :, None] + np.arange(KB, dtype=np.int16)[None, None]  # [A, NT, KB]
    acc_krc = np.ascontiguousarray(acc[n].transpose(1, 2, 0))  # [A, R, C] f32
    hi = acc_krc.astype(FP8)
    res = (acc_krc - hi.astype(np.float32)).astype(FP8)
    pair = np.concatenate([hi[..., None, :], res[..., None, :]], axis=2)  # [A,R,2,C]
    lt = pair[np.arange(A)[:, None, None], idx]  # [A, NT, KB, 2, C]
    lt = lt.reshape(SUPER, STACK, NT, KB, LW).transpose(2, 1, 3, 0, 4)
    return np.ascontiguousarray(lt.reshape(NT, K, SUPER, LW))


def _ctab(acc, n, yh):
    """Combined per-batch stream: [NT, NB, K, BW] fp8.

    Per (tile, batch): cols [0, G*LW) hold G lhsT hi/res blocks, cols
    [G*LW, BW) hold G one-hot rhs blocks.
    """
    lt = _ltab(acc, n, yh).reshape(NT, K, NB, G * LW).transpose(0, 2, 1, 3)
    st = _stab(yh).reshape(NT, K, NB, G * COLS).transpose(0, 2, 1, 3)
    return np.ascontiguousarray(np.concatenate([lt, st], axis=3))


def _build_nc():
    if "nc" in _cache:
        return _cache["nc"]
    nc = bass.Bass("TRN2", debug=False, target_bir_lowering=False, num_devices=NCORES)
    ctab_d = nc.dram_tensor(
        "ctab", [NT, NB, K, BW], mybir.dt.float8e4, kind="ExternalInput"
    ).ap()
    raw_d = nc.dram_tensor(
        "raw", [NT, C, COLS], mybir.dt.float32, kind="ExternalOutput"
    ).ap()

    ctx = ExitStack()
    _cache["ctx"] = ctx
    in_sb = [
        ctx.enter_context(nc.sbuf_tensor(f"in{i}", [K, BW], mybir.dt.float8e4))
        for i in range(RING)
    ]
    obuf = [
        ctx.enter_context(nc.sbuf_tensor(f"ob{i}", [C, COLS], mybir.dt.float32))
        for i in range(2)
    ]
    psum = [
        ctx.enter_context(nc.psum_tensor(f"ps{i}", [C, COLS], mybir.dt.float32))
        for i in range(4)
    ]
    in_sem = [ctx.enter_context(nc.semaphore(f"in_sem{i}")) for i in range(RING)]
    o_sem = [ctx.enter_context(nc.semaphore(f"o_sem{i}")) for i in range(2)]
    pe_sem = ctx.enter_context(nc.semaphore("pe_sem"))
    cp_sem = ctx.enter_context(nc.semaphore("cp_sem"))
    block = ctx.enter_context(nc.Block())

    @block.gpsimd
    def _(gpsimd):
        for j in range(NJ):
            t, b = divmod(j, NB)
            if j >= RING:
                # slot j%RING free once PE finished batch j-RING
                gpsimd.wait_ge(pe_sem, j - RING + 1)
            gpsimd.dma_start(in_sb[j % RING][:], ctab_d[t, b]).then_inc(
                in_sem[j % RING], 16
            )

    @block.tensor
    def _(tensor):
        for j in range(NJ):
            t, b = divmod(j, NB)
            tensor.wait_ge(in_sem[j % RING], 16 * (j // RING + 1))
            if b == 0 and t >= 4:
                tensor.wait_ge(cp_sem, t - 3)  # psum slot drained
            mm = None
            for kk in range(G):
                sk = b * G + kk
                lhsT = in_sb[j % RING][:, kk * LW : (kk + 1) * LW].rearrange(
                    "p (two m) -> p two m", two=2
                )
                for h in range(MMH):
                    off = G * LW + kk * COLS + h * 512
                    sel = in_sb[j % RING][:, off : off + 512]
                    # DoubleRow pair dim with stride 0: rhs = [Sel | Sel]
                    rhs = bass.AP(sel.tensor, sel.offset, [sel.ap[0], [0, 2], sel.ap[1]])
                    mm = tensor.matmul(
                        out=psum[t % 4][:, h * 512 : (h + 1) * 512],
                        lhsT=lhsT,
                        rhs=rhs,
                        perf_mode=mybir.MatmulPerfMode.DoubleRow,
                        start=(sk == 0),
                        stop=(sk == SUPER - 1),
                    )
            mm.then_inc(pe_sem, 1)

    @block.scalar
    def _(scalar):
        for t in range(NT):
            scalar.wait_ge(pe_sem, NB * (t + 1))
            if t >= 2:
                scalar.wait_ge(o_sem[t % 2], 16 * (t // 2))  # obuf slot free
            scalar.copy(obuf[t % 2][:], psum[t % 4][:]).then_inc(cp_sem, 1)

    @block.sync
    def _(sync):
        for t in range(NT):
            sync.wait_ge(cp_sem, t + 1)
            sync.dma_start(raw_d[t], obuf[t % 2][:]).then_inc(o_sem[t % 2], 16)

    _cache["nc"] = nc
    return nc


def _install_ntff_hook():
    """Provide the antenv.axon_hooks shim the image lacks, wiring the
    ctypes NTFF profiler from trn_agent_boot."""
    import sys
    import types

    if "antenv.axon_hooks" in sys.modules:
        return
    import antenv
    from trn_agent_boot.trn_boot import _ntff_profile_via_ctypes

    mod = types.ModuleType("antenv.axon_hooks")
    hook = _ntff_profile_via_ctypes("/opt/axon/libaxon_pjrt.so")
    mod.get_axon_ntff_profile_hook = lambda: hook
    mod.set_axon_ntff_profile_hook = lambda h: None
    sys.modules["antenv.axon_hooks"] = mod
    antenv.axon_hooks = mod


def hw_exec_time_ns(trace_cores=None):
    """Re-run the last kernel() invocation with tracing; return max core ns."""
    _install_ntff_hook()
    nc = _cache["nc"]
    res = run_bass_kernel_spmd(
        nc,
        _cache["in_maps"],
        core_ids=list(range(NCORES)),
        trace=True,
        trace_cores=trace_cores,
    )
    _cache["trace"] = res
    return res.exec_time_ns


def kernel(accumulator, out_H=128, out_W=128, numangle=180, numrho=184):
    accumulator = np.asarray(accumulator, np.float32)
    assert accumulator.shape == (N, C, A, R), accumulator.shape
    assert int(out_H) == H and int(out_W) == W
    assert int(numangle) == A and int(numrho) == R

    nc = _build_nc()
    in_maps = []
    for core in range(NCORES):
        n, yh = divmod(core, 2)
        in_maps.append({"ctab": _ctab(accumulator, n, yh)})
    _cache["in_maps"] = in_maps
    res = run_bass_kernel_spmd(nc, in_maps, core_ids=list(range(NCORES)))

    # Unshard: cores hold disjoint output slabs.
    out = np.empty((N, C, H, W), np.float32)
    for core in range(NCORES):
        n, yh = divmod(core, 2)
        raw = res.results[core]["raw"]  # [NT, C, COLS]
        slab = (
            raw.reshape(NTY, NTX, C, TS, TS)
            .transpose(2, 0, 3, 1, 4)
            .reshape(C, 64, W)
        )
        out[n, :, yh * 64 : (yh + 1) * 64, :] = slab
    return out


# revision 9
# speedup vs baseline: 18.5789x; 1.2738x over previous
"""Trainium2 Bass kernel for the inverse deep-hough-transform gather-reduce.

out[n, c, y, x] = sum_k acc[n, c, k, rho_idx[k, y, x]]

Design (v5): fp8 DoubleRow one-hot selection matmuls on the PE
--------------------------------------------------------------
For a 16x16 output tile and angle k, rho_idx spans a band of at most
15*(|sin|+|cos|)+2 <= 24 consecutive rho values.  The per-angle
gather-reduce over a tile is therefore a small-contraction matmul with a
0/1 one-hot selection matrix:

    psum[c, col] += sum_p acc[n, c, k, base_kt + p] * Sel_kt[p, col]

where Sel_kt[p, col] = 1[rho_idx(k, y(col), x(col)) - base_kt == p].
Five angles are stacked per matmul (K = 5*24 = 120 <= 128) and all 180
angles accumulate in PSUM (fp32).  The acc bands (lhsT) and the one-hot
tables (rhs) are layout-prepped host-side (pure static re-indexing of
the input + 0/1 tables) and streamed from HBM as one fp8 tensor; the
16x16 tiling minimizes streamed bytes (band rows per column).

fp8 DoubleRow: the DoubleRow pair dim carries hi = fp8(acc) and
res = fp8(acc - hi) against the same one-hot rhs (stride-0 pair dim),
so each matmul computes (hi + res).T @ Sel at one output column per
cycle -- quantization error ~1e-3.

Sharding: core = 2*n + yhalf (N=4 samples x 2 y-halves).  Each core
computes out[n, :, yh*64:(yh+1)*64, :] -- full inputs, disjoint outputs,
no cross-core reduction.  Per core: 32 tiles x 36 matmuls.

Sync note: a dma_start's completion semaphore gets +16 spread across the
DMA engines as sub-streams finish, so increments of back-to-back DMAs on
one semaphore interleave; waiting for 16*(j+1) on a shared semaphore
does NOT guarantee DMA j finished.  Each ring slot therefore gets its
own semaphore, and slot reuse is gated on the consumer (so increments of
different uses of one slot cannot overlap in time).
"""

from contextlib import ExitStack

import numpy as np
import ml_dtypes

import concourse.bass as bass
from concourse import mybir
from concourse.bass_utils import run_bass_kernel_spmd

# Problem constants (hardcoded per the harness contract).
N, C, A, R = 4, 128, 180, 184
H = W = 128
NCORES = 8

KB = 24  # band rows per angle (>= max band width 24)
STACK = 5  # angles stacked per matmul
K = KB * STACK  # 120 contraction rows
SUPER = A // STACK  # 36 superangles
TS = 16  # tile side
NTY, NTX = 4, 8  # tiles per core: 4 (y within half) x 8 (x)
NT = NTY * NTX  # 32 tiles
COLS = TS * TS  # 256 columns per tile
LW = 2 * C  # lhsT width per super: fp8 [hi(128) | res(128)]
BW = SUPER * (LW + COLS)  # combined tile width (18432 B/partition)
RING = 6  # input ring depth

FP8 = ml_dtypes.float8_e4m3

_cache = {}


def _r_table():
    """[A, H, W] int16 rho indices, exactly as the reference computes them."""
    if "rtab" in _cache:
        return _cache["rtab"]
    k = np.arange(A)
    theta = k * (np.pi / A)
    cos_t = np.cos(theta)
    sin_t = np.sin(theta)
    y, x = np.meshgrid(np.arange(H), np.arange(W), indexing="ij")
    xc = (x - W // 2).astype(np.float64)
    yc = (y - H // 2).astype(np.float64)
    r = np.round(cos_t[:, None, None] * xc[None] + sin_t[:, None, None] * yc[None])
    r = r.astype(np.int64) + R // 2
    assert (r >= 0).all() and (r < R).all()  # always valid for these shapes
    _cache["rtab"] = r.astype(np.int16)
    return _cache["rtab"]


def _bases():
    """[A, 8, 8] int16 band base per (angle, global ty, tx); width <= KB."""
    if "bases" in _cache:
        return _cache["bases"]
    r = _r_table().reshape(A, 8, TS, 8, TS)
    rmin = r.min(axis=(2, 4))  # [A, 8ty, 8tx]
    rmax = r.max(axis=(2, 4))
    assert int((rmax - rmin).max()) < KB, (rmax - rmin).max()
    base = np.minimum(rmin, R - KB).astype(np.int16)
    assert int((rmax - base).max()) < KB
    _cache["bases"] = base
    return _cache["bases"]


def _stab(yh):
    """One-hot rhs tables for y-half yh: [NT, K, SUPER, COLS] fp8.

    stab[t, j*KB+p, sk, col] = 1[ r(5sk+j, y, x) - base == p ]
    with t = ty*8+tx, y = yh*64+ty*16+(col//16), x = tx*16+(col%16).
    """
    key = ("stab", yh)
    if key in _cache:
        return _cache[key]
    r = _r_table()[:, yh * 64 : (yh + 1) * 64, :]  # [A, 64, 128]
    r = r.reshape(A, NTY, TS, NTX, TS).transpose(0, 1, 3, 2, 4).reshape(A, NT, COLS)
    base = _bases()[:, yh * 4 : (yh + 1) * 4, :].reshape(A, NT)  # [A, NT]
    rel = (r - base[:, :, None]).astype(np.int16)  # [A, NT, COLS] in [0, KB)
    onehot = rel[:, :, None, :] == np.arange(KB, dtype=np.int16)[None, None, :, None]
    # [A, NT, KB, COLS] -> [SUPER, STACK, NT, KB, COLS] -> [NT, STACK*KB, SUPER, COLS]
    onehot = onehot.reshape(SUPER, STACK, NT, KB, COLS).transpose(2, 1, 3, 0, 4)
    st = np.ascontiguousarray(onehot.reshape(NT, K, SUPER, COLS).astype(FP8))
    _cache[key] = st
    return st


def _ltab(acc, n, yh):
    """Band lhsT tables: [NT, K, SUPER, LW] fp8, DoubleRow pair layout.

    Per super: cols [0, C) = hi band, cols [C, 2C) = residual band, where
    hi = fp8(acc), res = fp8(acc - hi); the DoubleRow matmul computes
    hi.T @ Sel + res.T @ Sel = fp8-pair(acc).T @ Sel (~1e-3 quantization).

    ltab[t, j*KB+p, sk, {c, C+c}] = {hi, res}[n, c, 5sk+j, base_kt + p]
    """
    base = _bases()[:, yh * 4 : (yh + 1) * 4, :].reshape(A, NT)  # [A, NT]
    idx = base[:, :, None] + np.arange(KB, dtype=np.int16)[None, None]  # [A, NT, KB]
    acc_krc = np.ascontiguousarray(acc[n].transpose(1, 2, 0))  # [A, R, C] f32
    hi = acc_krc.astype(FP8)
    res = (acc_krc - hi.astype(np.float32)).astype(FP8)
    pair = np.concatenate([hi[..., None, :], res[..., None, :]], axis=2)  # [A,R,2,C]
    lt = pair[np.arange(A)[:, None, None], idx]  # [A, NT, KB, 2, C]
    lt = lt.reshape(SUPER, STACK, NT, KB, LW).transpose(2, 1, 3, 0, 4)
    return np.ascontiguousarray(lt.reshape(NT, K, SUPER, LW))


def _ctab(acc, n, yh):
    """Combined per-tile stream: [NT, K, BW] fp8.

    Per (tile, super): 512 B per partition -- lhsT hi/res block (256 B)
    then one-hot rhs block (256 B).
    """
    lt = _ltab(acc, n, yh)  # [NT, K, SUPER, LW]
    st = _stab(yh)  # [NT, K, SUPER, COLS]
    both = np.concatenate([lt, st], axis=3)  # [NT, K, SUPER, LW+COLS]
    return np.ascontiguousarray(both.reshape(NT, K, BW))


def _build_nc():
    if "nc" in _cache:
        return _cache["nc"]
    nc = bass.Bass("TRN2", debug=False, target_bir_lowering=False, num_devices=NCORES)
    ctab_d = nc.dram_tensor(
        "ctab", [NT, K, BW], mybir.dt.float8e4, kind="ExternalInput"
    ).ap()
    raw_d = nc.dram_tensor(
        "raw", [NT, C, COLS], mybir.dt.float32, kind="ExternalOutput"
    ).ap()

    ctx = ExitStack()
    _cache["ctx"] = ctx
    in_sb = [
        ctx.enter_context(nc.sbuf_tensor(f"in{i}", [K, BW], mybir.dt.float8e4))
        for i in range(RING)
    ]
    obuf = [
        ctx.enter_context(nc.sbuf_tensor(f"ob{i}", [C, COLS], mybir.dt.float32))
        for i in range(2)
    ]
    psum = [
        ctx.enter_context(nc.psum_tensor(f"ps{i}", [C, COLS], mybir.dt.float32))
        for i in range(4)
    ]
    in_sem = [ctx.enter_context(nc.semaphore(f"in_sem{i}")) for i in range(RING)]
    o_sem = [ctx.enter_context(nc.semaphore(f"o_sem{i}")) for i in range(2)]
    pe_sem = ctx.enter_context(nc.semaphore("pe_sem"))
    cp_sem = ctx.enter_context(nc.semaphore("cp_sem"))
    block = ctx.enter_context(nc.Block())

    @block.gpsimd
    def _(gpsimd):
        for t in range(NT):
            if t >= RING:
                # slot t%RING free once PE finished tile t-RING
                gpsimd.wait_ge(pe_sem, t - RING + 1)
            gpsimd.dma_start(in_sb[t % RING][:], ctab_d[t]).then_inc(
                in_sem[t % RING], 16
            )

    @block.tensor
    def _(tensor):
        for t in range(NT):
            tensor.wait_ge(in_sem[t % RING], 16 * (t // RING + 1))
            if t >= 4:
                tensor.wait_ge(cp_sem, t - 3)  # psum slot drained
            mm = None
            for sk in range(SUPER):
                lhsT = in_sb[t % RING][
                    :, sk * (LW + COLS) : sk * (LW + COLS) + LW
                ].rearrange("p (two m) -> p two m", two=2)
                sel = in_sb[t % RING][
                    :, sk * (LW + COLS) + LW : (sk + 1) * (LW + COLS)
                ]
                # DoubleRow pair dim with stride 0: rhs = [Sel | Sel]
                rhs = bass.AP(sel.tensor, sel.offset, [sel.ap[0], [0, 2], sel.ap[1]])
                mm = tensor.matmul(
                    out=psum[t % 4][:],
                    lhsT=lhsT,
                    rhs=rhs,
                    perf_mode=mybir.MatmulPerfMode.DoubleRow,
                    start=(sk == 0),
                    stop=(sk == SUPER - 1),
                )
            mm.then_inc(pe_sem, 1)

    @block.scalar
    def _(scalar):
        for t in range(NT):
            scalar.wait_ge(pe_sem, t + 1)
            if t >= 2:
                scalar.wait_ge(o_sem[t % 2], 16 * (t // 2))  # obuf slot free
            scalar.copy(obuf[t % 2][:], psum[t % 4][:]).then_inc(cp_sem, 1)

    @block.sync
    def _(sync):
        for t in range(NT):
            sync.wait_ge(cp_sem, t + 1)
            sync.dma_start(raw_d[t], obuf[t % 2][:]).then_inc(o_sem[t % 2], 16)

    _cache["nc"] = nc
    return nc


def _install_ntff_hook():
    """Provide the antenv.axon_hooks shim the image lacks, wiring the
    ctypes NTFF profiler from trn_agent_boot."""
    import sys
    import types

    if "antenv.axon_hooks" in sys.modules:
        return
    import antenv
    from trn_agent_boot.trn_boot import _ntff_profile_via_ctypes

    mod = types.ModuleType("antenv.axon_hooks")
    hook = _ntff_profile_via_ctypes("/opt/axon/libaxon_pjrt.so")
    mod.get_axon_ntff_profile_hook = lambda: hook
    mod.set_axon_ntff_profile_hook = lambda h: None
    sys.modules["antenv.axon_hooks"] = mod
    antenv.axon_hooks = mod


def hw_exec_time_ns(trace_cores=None):
    """Re-run the last kernel() invocation with tracing; return max core ns."""
    _install_ntff_hook()
    nc = _cache["nc"]
    res = run_bass_kernel_spmd(
        nc,
        _cache["in_maps"],
        core_ids=list(range(NCORES)),
        trace=True,
        trace_cores=trace_cores,
    )
    _cache["trace"] = res
    return res.exec_time_ns


def kernel(accumulator, out_H=128, out_W=128, numangle=180, numrho=184):
    accumulator = np.asarray(accumulator, np.float32)
    assert accumulator.shape == (N, C, A, R), accumulator.shape
    assert int(out_H) == H and int(out_W) == W
    assert int(numangle) == A and int(numrho) == R

    nc = _build_nc()
    in_maps = []
    for core in range(NCORES):
        n, yh = divmod(core, 2)
        in_maps.append({"ctab": _ctab(accumulator, n, yh)})
    _cache["in_maps"] = in_maps
    res = run_bass_kernel_spmd(nc, in_maps, core_ids=list(range(NCORES)))

    # Unshard: cores hold disjoint output slabs.
    out = np.empty((N, C, H, W), np.float32)
    for core in range(NCORES):
        n, yh = divmod(core, 2)
        raw = res.results[core]["raw"]  # [NT, C, COLS]
        slab = (
            raw.reshape(NTY, NTX, C, TS, TS)
            .transpose(2, 0, 3, 1, 4)
            .reshape(C, 64, W)
        )
        out[n, :, yh * 64 : (yh + 1) * 64, :] = slab
    return out
